# revision 6
# baseline (speedup 1.0000x reference)
"""Trainium2 Bass kernel for nn_EnsembleRSSM (DreamerV2-style RSSM).

Strategy:
- Pure data parallelism: B=256 -> 32 per core; obs scan (32) + imagine scan (32)
  fused into a 64-wide batch so every weight stream is shared by both scans.
- Activations batch-major [64, feat]; matmuls use transposed activations as the
  PE stationary operand and weights as the moving operand (bf16, 1 cyc/row).
- fp32 precision recovered via hi/lo bf16 split: X@W ~= Xh@Wh + Xl@Wh + Xh@Wl,
  accumulated natively in PSUM (host-sim measured rel err ~8e-6 vs fp32 ref,
  zero argmax flips).
- Categorical sampling: gumbel noise precomputed on host (exact jax bits);
  device does argmax via segmented reduce_max + is_equal -> one-hot.
- emb@W2b and act-part of W1 handled on host / via a 19-row K-block; biases
  folded in as ones-rows of the stationaries.
- kl_loss / kl_value computed on host from the returned logits.
"""
import sys
sys.path.insert(0, "/opt/trn_rl_repo")

import numpy as np
import ml_dtypes

import concourse.bacc as bacc
import concourse.tile as tile
import concourse.mybir as mybir
from concourse.bass_utils import run_bass_kernel_spmd

f32 = mybir.dt.float32
bf16 = mybir.dt.bfloat16
AF = mybir.ActivationFunctionType
ALU = mybir.AluOpType

B, T, EMB, ACTD = 256, 64, 1536, 18
S, D, M = 32, 32, 600
NCORES = 8
BL = B // NCORES          # 32 obs samples per core
BF = 2 * BL               # 64 = obs + img fused batch
SD = S * D                # 1024


def _kblocks(K):
    return (K + 127) // 128


def _chunks(n, c=512):
    o, out = 0, []
    while o < n:
        out.append((o, min(c, n - o)))
        o += c
    return out


def build_kernel(Tb=T):
    nc = bacc.Bacc()
    din = {}

    def dram_in(name, shape, dt=f32):
        din[name] = nc.dram_tensor(name, list(shape), dt, kind="ExternalInput")

    dram_in("w1_h", (128, 9 * 600), bf16);  dram_in("w1_l", (128, 9 * 600), bf16)
    dram_in("gd_h", (128, 5 * 1800), bf16); dram_in("gd_l", (128, 5 * 1800), bf16)
    dram_in("gx_h", (128, 5 * 1800), bf16); dram_in("gx_l", (128, 5 * 1800), bf16)
    dram_in("we1_h", (128, 5 * 600), bf16); dram_in("we1_l", (128, 5 * 600), bf16)
    dram_in("we2_h", (128, 5 * 1024), bf16); dram_in("we2_l", (128, 5 * 1024), bf16)
    dram_in("w3_h", (128, 5 * 1024), bf16);  dram_in("w3_l", (128, 5 * 1024), bf16)
    dram_in("w2d_h", (128, 5 * 600), bf16);  dram_in("w2d_l", (128, 5 * 600), bf16)
    dram_in("act_h", (Tb, 19, BF), bf16); dram_in("act_l", (Tb, 19, BF), bf16)
    dram_in("mask", (Tb, BF, 1))
    dram_in("g_pri", (Tb, BF, SD))
    dram_in("g_pos", (Tb, BL, SD))
    dram_in("epre", (Tb, BL, M))
    dram_in("stoch0", (BF, SD))
    dram_in("det0", (BF, M))
    dram_in("eye", (BF, BF))

    outs = {
        "lp_o": nc.dram_tensor("lp_o", [Tb, BF, SD], f32, kind="ExternalOutput"),
        "sp_o": nc.dram_tensor("sp_o", [Tb, BF, SD], f32, kind="ExternalOutput"),
        "lq_o": nc.dram_tensor("lq_o", [Tb, BL, SD], f32, kind="ExternalOutput"),
        "sq_o": nc.dram_tensor("sq_o", [Tb, BL, SD], f32, kind="ExternalOutput"),
        "det_o": nc.dram_tensor("det_o", [Tb, BF, M], f32, kind="ExternalOutput"),
    }
    with tile.TileContext(nc) as tc:
        _body(nc, tc, din, outs, Tb)
    nc.compile()
    return nc


def _body(nc, tc, din, outs, Tb):
    lp_o, sp_o, lq_o, sq_o, det_o = (outs[k] for k in
                                     ("lp_o", "sp_o", "lq_o", "sq_o", "det_o"))
    with tc.tile_pool(name="persist", bufs=1) as persist, \
         tc.tile_pool(name="wstage", bufs=12) as wstage, \
         tc.tile_pool(name="gstage", bufs=2) as gstage, \
         tc.tile_pool(name="actp", bufs=1) as actp, \
         tc.tile_pool(name="psum", bufs=1, space="PSUM") as psum:

        w1_h = persist.tile([128, 9 * 600], bf16, name="w1_h")
        w1_l = persist.tile([128, 9 * 600], bf16, name="w1_l")
        gd_h = persist.tile([128, 5 * 1800], bf16, name="gd_h")
        gd_l = persist.tile([128, 5 * 1800], bf16, name="gd_l")
        gx_h = persist.tile([128, 5 * 1800], bf16, name="gx_h")
        gx_l = persist.tile([128, 5 * 1800], bf16, name="gx_l")
        we1_h = persist.tile([128, 5 * 600], bf16, name="we1_h")
        we1_l = persist.tile([128, 5 * 600], bf16, name="we1_l")
        eye_s = persist.tile([BF, BF], f32)
        for nm, t_ in [("w1_h", w1_h), ("w1_l", w1_l), ("gd_h", gd_h), ("gd_l", gd_l),
                       ("gx_h", gx_h), ("gx_l", gx_l), ("we1_h", we1_h), ("we1_l", we1_l),
                       ("eye", eye_s)]:
            nc.sync.dma_start(t_[:], din[nm].ap())

        # transposed stationaries, rewritten in place each step
        st_h = persist.tile([128, 8 * BF], bf16, name="st_h")
        st_l = persist.tile([128, 8 * BF], bf16, name="st_l")
        dt_h = persist.tile([128, 5 * BF], bf16, name="dt_h")
        dt_l = persist.tile([128, 5 * BF], bf16, name="dt_l")
        xt_h = persist.tile([128, 5 * BF], bf16, name="xt_h")
        xt_l = persist.tile([128, 5 * BF], bf16, name="xt_l")
        nd_h = persist.tile([128, 5 * BF], bf16, name="nd_h")
        nd_l = persist.tile([128, 5 * BF], bf16, name="nd_l")
        ht_h = persist.tile([128, 5 * BF], bf16, name="ht_h")
        ht_l = persist.tile([128, 5 * BF], bf16, name="ht_l")
        xq_h = persist.tile([128, 5 * BL], bf16, name="xq_h")
        xq_l = persist.tile([128, 5 * BL], bf16, name="xq_l")

        def triple(ps, stat_h, stat_l, mov_h, mov_l, start, stop, skip_lo=False):
            nc.tensor.matmul(ps, stat_h, mov_h, start=start, stop=False)
            if not skip_lo:
                nc.tensor.matmul(ps, stat_l, mov_h, start=False, stop=False)
            nc.tensor.matmul(ps, stat_h, mov_l, start=False, stop=stop)

        def elu(out, src, rows, width):
            # min computed before max so `out` may alias `src`
            vm = actp.tile([rows, width], f32, tag="elu_v", name="vm")
            we = actp.tile([rows, width], f32, tag="elu_w", name="we")
            nc.vector.tensor_scalar_min(vm[0:rows, 0:width], src, 0.0)
            nc.vector.tensor_scalar_max(out, src, 0.0)
            nc.scalar.activation(we[0:rows, 0:width], vm[0:rows, 0:width], AF.Exp)
            nc.vector.scalar_tensor_tensor(out, out, -1.0, we[0:rows, 0:width],
                                           op0=ALU.add, op1=ALU.add)

        def transpose_hilo(src, rows, K, out_h, out_l, colw, with_lo=True):
            nblk = _kblocks(K)
            pst = psum.tile([128, 512], f32, tag="tr", bufs=1, name="pst")
            for kb in range(nblk):
                kr = min(128, K - kb * 128)
                nc.tensor.transpose(pst[0:kr, kb * colw:kb * colw + rows],
                                    src[0:rows, kb * 128:kb * 128 + kr],
                                    eye_s[0:rows, 0:rows])
            for kb in range(nblk):
                kr = min(128, K - kb * 128)
                p = pst[0:kr, kb * colw:kb * colw + rows]
                h = out_h[0:kr, kb * colw:kb * colw + rows]
                nc.scalar.copy(h, p)
                if with_lo:
                    nc.vector.tensor_tensor(out_l[0:kr, kb * colw:kb * colw + rows],
                                            p, h, op=ALU.subtract)

        def onehot(a_tile, rows, tag):
            mx = actp.tile([rows, S], f32, tag=f"mx{tag}", name="mx")
            a3 = a_tile[0:rows, :].rearrange("b (s d) -> b s d", d=D)
            nc.vector.reduce_max(mx[0:rows, :], a3, axis=mybir.AxisListType.X)
            oh = actp.tile([rows, SD], f32, tag=f"oh{tag}", name="oh")
            oh3 = oh[0:rows, :].rearrange("b (s d) -> b s d", d=D)
            mb = mx[0:rows, :].unsqueeze(2).broadcast_to([rows, S, D])
            nc.vector.tensor_tensor(oh3, a3, mb, op=ALU.is_equal)
            return oh

        def wchunk(name, kr, c0, cw):
            w = wstage.tile([128, 512], bf16, tag="wst", name="wst")
            nc.sync.dma_start(w[0:kr, 0:cw], din[name].ap()[0:kr, c0:c0 + cw])
            return w[0:kr, 0:cw]

        stoch_bm = None
        det_bm = None
        for t in range(Tb):
            g_pri = gstage.tile([BF, SD], f32, tag="g_pri")
            nc.sync.dma_start(g_pri[:], din["g_pri"].ap()[t])
            g_pos = gstage.tile([BL, SD], f32, tag="g_pos")
            nc.sync.dma_start(g_pos[:], din["g_pos"].ap()[t])
            epre = gstage.tile([BL, M], f32, tag="epre")
            nc.sync.dma_start(epre[:], din["epre"].ap()[t])
            msk = gstage.tile([BF, 1], f32, tag="msk")
            nc.sync.dma_start(msk[:], din["mask"].ap()[t])
            a_h = gstage.tile([19, BF], bf16, tag="a_h")
            nc.sync.dma_start(a_h[:], din["act_h"].ap()[t])
            a_l = gstage.tile([19, BF], bf16, tag="a_l")
            nc.sync.dma_start(a_l[:], din["act_l"].ap()[t])

            if t == 0:
                stoch_bm = actp.tile([BF, SD], f32, tag="stoch", bufs=2)
                nc.sync.dma_start(stoch_bm[:], din["stoch0"].ap())
                det_bm = actp.tile([BF, M + 1], f32, tag="det", bufs=2)
                nc.sync.dma_start(det_bm[:, 0:M], din["det0"].ap())
                nc.vector.memset(det_bm[:, M:M + 1], 1.0)

            # mask carries in place, then build stationaries
            nc.vector.tensor_scalar_mul(stoch_bm[:], stoch_bm[:], msk[:, 0:1])
            nc.vector.tensor_scalar_mul(det_bm[:], det_bm[:], msk[:, 0:1])
            transpose_hilo(stoch_bm, BF, SD, st_h, st_l, BF, with_lo=(t == 0))
            transpose_hilo(det_bm, BF, M, dt_h, dt_l, BF)

            # x = elu([stoch, act, 1] @ W1b)
            x_ps = psum.tile([BF, 600], f32, tag="mm", bufs=2)
            for co, cw in _chunks(600):
                for kb in range(8):
                    triple(x_ps[:, co:co + cw],
                           st_h[:, kb * BF:(kb + 1) * BF], st_l[:, kb * BF:(kb + 1) * BF],
                           w1_h[:, kb * 600 + co: kb * 600 + co + cw],
                           w1_l[:, kb * 600 + co: kb * 600 + co + cw],
                           start=(kb == 0), stop=False, skip_lo=(t != 0))
                triple(x_ps[:, co:co + cw], a_h[:], a_l[:],
                       w1_h[0:19, 8 * 600 + co: 8 * 600 + co + cw],
                       w1_l[0:19, 8 * 600 + co: 8 * 600 + co + cw],
                       start=False, stop=True)
            x_bm = actp.tile([BF, M], f32, tag="x_bm")
            elu(x_bm[:], x_ps[:], BF, 600)
            transpose_hilo(x_bm, BF, M, xt_h, xt_l, BF)

            # GRU
            rz_ps = psum.tile([BF, 1200], f32, tag="rz", bufs=1)
            for co, cw in _chunks(1200):
                for kb in range(5):
                    kr = min(128, 601 - kb * 128)
                    triple(rz_ps[:, co:co + cw],
                           dt_h[0:kr, kb * BF:(kb + 1) * BF], dt_l[0:kr, kb * BF:(kb + 1) * BF],
                           gd_h[0:kr, kb * 1800 + co: kb * 1800 + co + cw],
                           gd_l[0:kr, kb * 1800 + co: kb * 1800 + co + cw],
                           start=(kb == 0), stop=False)
                for kb in range(5):
                    kr = min(128, 600 - kb * 128)
                    triple(rz_ps[:, co:co + cw],
                           xt_h[0:kr, kb * BF:(kb + 1) * BF], xt_l[0:kr, kb * BF:(kb + 1) * BF],
                           gx_h[0:kr, kb * 1800 + co: kb * 1800 + co + cw],
                           gx_l[0:kr, kb * 1800 + co: kb * 1800 + co + cw],
                           start=False, stop=(kb == 4))
            n_ps = psum.tile([BF, 600], f32, tag="mm", bufs=2)
            for co, cw in _chunks(600):
                for kb in range(5):
                    kr = min(128, 601 - kb * 128)
                    triple(n_ps[:, co:co + cw],
                           dt_h[0:kr, kb * BF:(kb + 1) * BF], dt_l[0:kr, kb * BF:(kb + 1) * BF],
                           gd_h[0:kr, kb * 1800 + 1200 + co: kb * 1800 + 1200 + co + cw],
                           gd_l[0:kr, kb * 1800 + 1200 + co: kb * 1800 + 1200 + co + cw],
                           start=(kb == 0), stop=(kb == 4))
            nx_ps = psum.tile([BF, 600], f32, tag="mm", bufs=2)
            for co, cw in _chunks(600):
                for kb in range(5):
                    kr = min(128, 600 - kb * 128)
                    triple(nx_ps[:, co:co + cw],
                           xt_h[0:kr, kb * BF:(kb + 1) * BF], xt_l[0:kr, kb * BF:(kb + 1) * BF],
                           gx_h[0:kr, kb * 1800 + 1200 + co: kb * 1800 + 1200 + co + cw],
                           gx_l[0:kr, kb * 1800 + 1200 + co: kb * 1800 + 1200 + co + cw],
                           start=(kb == 0), stop=(kb == 4))

            rz_bm = actp.tile([BF, 1200], f32, tag="rz_bm")
            nc.scalar.activation(rz_bm[:], rz_ps[:], AF.Sigmoid)
            gt = actp.tile([BF, M], f32, tag="gt")
            nc.vector.tensor_tensor(gt[:], rz_bm[:, 0:600], nx_ps[:], op=ALU.mult)
            nc.vector.tensor_tensor(gt[:], gt[:], n_ps[:], op=ALU.add)
            n_bm = actp.tile([BF, M], f32, tag="n_bm")
            nc.scalar.activation(n_bm[:], gt[:], AF.Tanh)
            nc.vector.tensor_tensor(gt[:], x_bm[:], n_bm[:], op=ALU.subtract)
            nc.vector.tensor_tensor(gt[:], rz_bm[:, 600:1200], gt[:], op=ALU.mult)
            ndet = actp.tile([BF, M + 1], f32, tag="det", bufs=2)
            nc.vector.tensor_tensor(ndet[:, 0:M], n_bm[:], gt[:], op=ALU.add)
            nc.vector.memset(ndet[:, M:M + 1], 1.0)
            nc.sync.dma_start(det_o.ap()[t], ndet[:, 0:M])
            transpose_hilo(ndet, BF, M + 1, nd_h, nd_l, BF)

            # prior head
            h_ps = psum.tile([BF, 600], f32, tag="mm", bufs=2)
            for co, cw in _chunks(600):
                for kb in range(5):
                    kr = min(128, 601 - kb * 128)
                    triple(h_ps[:, co:co + cw],
                           nd_h[0:kr, kb * BF:(kb + 1) * BF], nd_l[0:kr, kb * BF:(kb + 1) * BF],
                           we1_h[0:kr, kb * 600 + co: kb * 600 + co + cw],
                           we1_l[0:kr, kb * 600 + co: kb * 600 + co + cw],
                           start=(kb == 0), stop=(kb == 4))
            h_bm = actp.tile([BF, M + 1], f32, tag="h_bm")
            elu(h_bm[:, 0:M], h_ps[:], BF, 600)
            nc.vector.memset(h_bm[:, M:M + 1], 1.0)
            transpose_hilo(h_bm, BF, M + 1, ht_h, ht_l, BF)

            lp_ps = psum.tile([BF, SD], f32, tag="mm", bufs=2)
            for co, cw in _chunks(SD):
                for kb in range(5):
                    kr = min(128, 601 - kb * 128)
                    wh = wchunk("we2_h", kr, kb * 1024 + co, cw)
                    wl = wchunk("we2_l", kr, kb * 1024 + co, cw)
                    triple(lp_ps[:, co:co + cw],
                           ht_h[0:kr, kb * BF:(kb + 1) * BF], ht_l[0:kr, kb * BF:(kb + 1) * BF],
                           wh, wl, start=(kb == 0), stop=(kb == 4))
            lp_bm = actp.tile([BF, SD], f32, tag="lp_bm")
            nc.scalar.copy(lp_bm[:], lp_ps[:])
            nc.sync.dma_start(lp_o.ap()[t], lp_bm[:])
            a_pri = actp.tile([BF, SD], f32, tag="ga")
            nc.vector.tensor_tensor(a_pri[:], lp_ps[:], g_pri[:], op=ALU.add)
            oh_pri = onehot(a_pri, BF, "p")
            nc.sync.dma_start(sp_o.ap()[t], oh_pri[:])

            # posterior head
            q_ps = psum.tile([BF, 600], f32, tag="mm", bufs=2)
            for co, cw in _chunks(600):
                for kb in range(5):
                    kr = min(128, 601 - kb * 128)
                    wh = wchunk("w2d_h", kr, kb * 600 + co, cw)
                    wl = wchunk("w2d_l", kr, kb * 600 + co, cw)
                    triple(q_ps[:, co:co + cw],
                           nd_h[0:kr, kb * BF:(kb + 1) * BF], nd_l[0:kr, kb * BF:(kb + 1) * BF],
                           wh, wl, start=(kb == 0), stop=(kb == 4))
            xp_bm = actp.tile([BL, M + 1], f32, tag="xp_bm")
            nc.vector.tensor_tensor(xp_bm[:, 0:M], q_ps[0:BL, :], epre[:], op=ALU.add)
            elu(xp_bm[:, 0:M], xp_bm[:, 0:M], BL, 600)
            nc.vector.memset(xp_bm[:, M:M + 1], 1.0)
            transpose_hilo(xp_bm, BL, M + 1, xq_h, xq_l, BL)

            lq_ps = psum.tile([BL, SD], f32, tag="mm", bufs=2)
            for co, cw in _chunks(SD):
                for kb in range(5):
                    kr = min(128, 601 - kb * 128)
                    wh = wchunk("w3_h", kr, kb * 1024 + co, cw)
                    wl = wchunk("w3_l", kr, kb * 1024 + co, cw)
                    triple(lq_ps[:, co:co + cw],
                           xq_h[0:kr, kb * BL:(kb + 1) * BL], xq_l[0:kr, kb * BL:(kb + 1) * BL],
                           wh, wl, start=(kb == 0), stop=(kb == 4))
            lq_bm = actp.tile([BL, SD], f32, tag="lq_bm")
            nc.scalar.copy(lq_bm[:], lq_ps[:])
            nc.sync.dma_start(lq_o.ap()[t], lq_bm[:])
            a_pos = actp.tile([BL, SD], f32, tag="ga2")
            nc.vector.tensor_tensor(a_pos[:], lq_ps[:], g_pos[:], op=ALU.add)
            oh_pos = onehot(a_pos, BL, "q")
            nc.sync.dma_start(sq_o.ap()[t], oh_pos[:])

            if t < Tb - 1:
                stoch_next = actp.tile([BF, SD], f32, tag="stoch", bufs=2)
                nc.vector.tensor_copy(stoch_next[0:BL, :], oh_pos[0:BL, :])
                nc.vector.tensor_copy(stoch_next[BL:BF, :], oh_pri[BL:BF, :])
                stoch_bm = stoch_next
                det_bm = ndet


# ---------------- host side ----------------

_CACHE = {}


def _get_kernel(Tb=T):
    if Tb not in _CACHE:
        _CACHE[Tb] = build_kernel(Tb)
    return _CACHE[Tb]


def _bfsplit(w):
    h = w.astype(ml_dtypes.bfloat16)
    l = (w - h.astype(np.float32)).astype(ml_dtypes.bfloat16)
    return h, l


def _pack_w(w, N):
    """w [K, N] f32 -> hi/lo [128, nblk*N] bf16, k-major blocks, zero-padded."""
    K = w.shape[0]
    nblk = _kblocks(K)
    h, l = _bfsplit(np.asarray(w, np.float32))
    out_h = np.zeros((128, nblk * N), ml_dtypes.bfloat16)
    out_l = np.zeros((128, nblk * N), ml_dtypes.bfloat16)
    for kb in range(nblk):
        kr = min(128, K - kb * 128)
        out_h[0:kr, kb * N:(kb + 1) * N] = h[kb * 128:kb * 128 + kr]
        out_l[0:kr, kb * N:(kb + 1) * N] = l[kb * 128:kb * 128 + kr]
    return out_h, out_l


def _gumbel_rngs(Tb=T):
    import jax
    import jax.numpy as jnp
    cpu = jax.devices("cpu")[0]
    with jax.default_device(cpu):
        base = jax.random.key(123)
        gum = jax.jit(lambda k: jax.random.gumbel(k, (B, S, D), jnp.float32))
        def gf(idxs):
            return np.stack([np.asarray(gum(jax.random.fold_in(base, i))) for i in idxs])
        g_prior = gf([2 * t for t in range(Tb)])
        g_post = gf([2 * t + 1 for t in range(Tb)])
        g_img = gf([3 * T + t for t in range(Tb)])
        k0 = jax.random.fold_in(base, 10 ** 6)
        stoch0 = np.asarray(jax.random.uniform(k0, (B, S, D), jnp.float32))
    return g_prior, g_post, g_img, stoch0


def _kl_host(post_stats, prior_stats):
    p = post_stats.astype(np.float32)
    q = prior_stats.astype(np.float32)

    def logsm(x):
        m = x.max(-1, keepdims=True)
        e = np.exp(x - m, dtype=np.float32)
        return (x - m) - np.log(e.sum(-1, keepdims=True, dtype=np.float32),
                                dtype=np.float32)

    lp_, lq_ = logsm(p), logsm(q)
    sm = np.exp(lp_, dtype=np.float32)
    kl_value = (sm * (lp_ - lq_)).sum(-1, dtype=np.float32).sum(-1, dtype=np.float32)
    mn = np.float32(max(kl_value.mean(dtype=np.float64), 1.0))
    kl_loss = np.float32(np.float32(0.8) * mn + np.float32(0.2) * mn)
    return kl_loss, kl_value.astype(np.float32)


def kernel(embed, action, is_first, W1, b1, Wir, bir, Wiz, biz, Win, bin_,
           Whr, Whz, Whn, We1, be1, We2, be2, W2, b2, W3, b3, _Tb=T, _trace=False):
    embed = np.asarray(embed, np.float32)
    action = np.asarray(action, np.float32)
    is_first = np.asarray(is_first, np.float32)
    Wn = {k: np.asarray(v, np.float32) for k, v in dict(
        W1=W1, b1=b1, Wir=Wir, bir=bir, Wiz=Wiz, biz=biz, Win=Win, bin_=bin_,
        Whr=Whr, Whz=Whz, Whn=Whn, We1=We1, be1=be1, We2=We2, be2=be2,
        W2=W2, b2=b2, W3=W3, b3=b3).items()}
    Tb = _Tb

    nc = _get_kernel(Tb)
    g_prior, g_post, g_img, stoch0 = _gumbel_rngs(Tb)

    # ---- pack weights (shared across cores) ----
    w1b = np.zeros((1043, 600), np.float32)
    w1b[0:SD + ACTD] = Wn["W1"]
    w1b[SD + ACTD] = Wn["b1"]
    w1_h, w1_l = _pack_w(w1b, 600)
    gdet = np.zeros((601, 1800), np.float32)
    gdet[0:600, 0:600] = Wn["Wir"]; gdet[0:600, 600:1200] = Wn["Wiz"]
    gdet[0:600, 1200:1800] = Wn["Win"]
    gdet[600, 0:600] = Wn["bir"]; gdet[600, 600:1200] = Wn["biz"]
    gdet[600, 1200:1800] = Wn["bin_"]
    gd_h, gd_l = _pack_w(gdet, 1800)
    gxw = np.concatenate([Wn["Whr"], Wn["Whz"], Wn["Whn"]], axis=1)
    gx_h, gx_l = _pack_w(gxw, 1800)
    we1b = np.zeros((601, 600), np.float32)
    we1b[0:600] = Wn["We1"]; we1b[600] = Wn["be1"]
    we1_h, we1_l = _pack_w(we1b, 600)
    we2b = np.zeros((601, 1024), np.float32)
    we2b[0:600] = Wn["We2"]; we2b[600] = Wn["be2"]
    we2_h, we2_l = _pack_w(we2b, 1024)
    w2db = np.zeros((601, 600), np.float32)
    w2db[0:600] = Wn["W2"][0:600]; w2db[600] = Wn["b2"]
    w2d_h, w2d_l = _pack_w(w2db, 600)
    w3b = np.zeros((601, 1024), np.float32)
    w3b[0:600] = Wn["W3"]; w3b[600] = Wn["b3"]
    w3_h, w3_l = _pack_w(w3b, 1024)

    # emb @ W2b on host (fp32)
    embpre_all = np.dot(embed.reshape(-1, EMB), Wn["W2"][600:]).astype(np.float32)
    embpre_all = embpre_all.reshape(B, T, M)

    eye = np.eye(BF, dtype=np.float32)
    shared = dict(w1_h=w1_h, w1_l=w1_l, gd_h=gd_h, gd_l=gd_l, gx_h=gx_h, gx_l=gx_l,
                  we1_h=we1_h, we1_l=we1_l, we2_h=we2_h, we2_l=we2_l,
                  w3_h=w3_h, w3_l=w3_l, w2d_h=w2d_h, w2d_l=w2d_l, eye=eye)

    in_maps = []
    for c in range(NCORES):
        sl = slice(c * BL, (c + 1) * BL)
        m = (1.0 - is_first[sl]).astype(np.float32)            # [BL, T]
        mask = np.concatenate([m.T[:, :, None],
                               np.ones((T, BL, 1), np.float32)], axis=1)[:Tb]  # [T, BF, 1]
        act_c = action[sl]                                      # [BL, T, 18]
        act_obs = (act_c * m[:, :, None]).transpose(1, 2, 0)    # [T, 18, BL]
        act_img = act_c.transpose(1, 2, 0)                      # [T, 18, BL]
        actT = np.zeros((T, 19, BF), np.float32)
        actT[:, 0:ACTD, 0:BL] = act_obs
        actT[:, 0:ACTD, BL:BF] = act_img
        actT[:, ACTD, :] = 1.0
        a_h, a_l = _bfsplit(actT[:Tb])
        st0 = stoch0[sl].reshape(BL, SD)
        stoch0_c = np.concatenate([st0, st0], axis=0)            # [BF, SD]
        gp = np.concatenate([g_prior[:, sl].reshape(Tb, BL, SD),
                             g_img[:, sl].reshape(Tb, BL, SD)], axis=1)  # [T, BF, SD]
        gq = g_post[:, sl].reshape(Tb, BL, SD)
        epre = np.ascontiguousarray(embpre_all[sl].transpose(1, 0, 2)[:Tb])  # [T, BL, M]
        in_maps.append(dict(shared,
                            act_h=a_h, act_l=a_l, mask=np.ascontiguousarray(mask),
                            g_pri=np.ascontiguousarray(gp), g_pos=np.ascontiguousarray(gq),
                            epre=epre, stoch0=stoch0_c,
                            det0=np.zeros((BF, M), np.float32)))

    res = run_bass_kernel_spmd(nc, in_maps, core_ids=list(range(NCORES)),
                               trace=_trace, trace_cores=[0] if _trace else None)

    # ---- assemble outputs ----
    lp = np.zeros((B, Tb, S, D), np.float32); sp = np.zeros((B, Tb, S, D), np.float32)
    lq = np.zeros((B, Tb, S, D), np.float32); sq = np.zeros((B, Tb, S, D), np.float32)
    dt = np.zeros((B, Tb, M), np.float32)
    ilp = np.zeros((B, Tb, S, D), np.float32); isp = np.zeros((B, Tb, S, D), np.float32)
    idt = np.zeros((B, Tb, M), np.float32)
    for c in range(NCORES):
        sl = slice(c * BL, (c + 1) * BL)
        r = res.results[c]
        lp[sl] = r["lp_o"][:, 0:BL].transpose(1, 0, 2).reshape(BL, Tb, S, D)
        ilp[sl] = r["lp_o"][:, BL:BF].transpose(1, 0, 2).reshape(BL, Tb, S, D)
        sp[sl] = r["sp_o"][:, 0:BL].transpose(1, 0, 2).reshape(BL, Tb, S, D)
        isp[sl] = r["sp_o"][:, BL:BF].transpose(1, 0, 2).reshape(BL, Tb, S, D)
        lq[sl] = r["lq_o"].transpose(1, 0, 2).reshape(BL, Tb, S, D)
        sq[sl] = r["sq_o"].transpose(1, 0, 2).reshape(BL, Tb, S, D)
        dt[sl] = r["det_o"][:, 0:BL].transpose(1, 0, 2)
        idt[sl] = r["det_o"][:, BL:BF].transpose(1, 0, 2)

    kl_loss, kl_value = _kl_host(lq, lp)
    out = (lq, sq, dt, lp, sp, kl_loss, kl_value, ilp, isp, idt)
    if _trace:
        return out, res
    return out
